# revision 1
# baseline (speedup 1.0000x reference)
import numpy as np
import ml_dtypes

import concourse.bass as bass
import concourse.tile as tile
from concourse import mybir, bacc
from concourse.masks import make_identity

L, B, EMB, REC = 128, 32, 512, 128
VOCAB = 50257
NCORES = 8
VS = 6283
VPAD = VS * NCORES
PAD_COLS = VPAD - VOCAB
NPOS = L * B
NTILE = NPOS // 128
NPB = 32
EWIDTH = 1024
NVT = 7
LAST_W = VS - (NVT - 1) * EWIDTH
E2WIDTH = 2048
NVT2 = 4
LAST_W2 = VS - (NVT2 - 1) * E2WIDTH
OUT_BF16 = True
INTERLEAVE_P1 = True

SCH_A = float(np.float32(2.0**23 / np.log(2.0)))
SCH_B = float(np.float32((127 << 23) - 482619))
PADEXP = float(np.int32(SCH_B).view(np.float32))

BF = mybir.dt.bfloat16
F32 = mybir.dt.float32
I32 = mybir.dt.int32
AF = mybir.ActivationFunctionType
ALU = mybir.AluOpType

B_RF, B_IF, B_RB, B_IB, B_NF, B_NB, B2NF, B2NB = range(8)


def build(phases=("front", "rec", "pass1", "ar", "pass2")):
    nc = bacc.Bacc(num_swdge_queues=4)

    idx_p = nc.declare_dram_parameter("idx", [128, NTILE], I32, isOutput=False)
    emb_p = nc.declare_dram_parameter("emb", [VOCAB, EMB], BF, isOutput=False)
    ut_p = nc.declare_dram_parameter("ut", [EMB, 768], BF, isOutput=False)
    wt_p = nc.declare_dram_parameter("wt", [REC, 768], F32, isOutput=False)
    bias_p = nc.declare_dram_parameter("bias", [128, 8], F32, isOutput=False)
    b2n_p = nc.declare_dram_parameter("b2nrow", [64, 128], F32, isOutput=False)
    vt_p = nc.declare_dram_parameter("vt", [2 * REC, VS], BF, isOutput=False)
    ib_p = nc.declare_dram_parameter("ib", [128, B], BF, isOutput=False)
    bcri_p = nc.declare_dram_parameter("bcri", [128, 512], BF, isOutput=False)
    out_dt = BF if OUT_BF16 else F32
    out_p = nc.declare_dram_parameter("out", [NPOS, VS], out_dt, isOutput=True)
    nls_p = nc.declare_dram_parameter("nls", [128, NPB], F32, isOutput=True)

    cc_inA = nc.dram_tensor("cc_inA", [128, 22], F32)
    cc_outA = nc.dram_tensor("cc_outA", [128, 22], F32)
    cc_inB = nc.dram_tensor("cc_inB", [128, 10], F32)
    cc_outB = nc.dram_tensor("cc_outB", [128, 10], F32)

    with tile.TileContext(nc) as tc:
        from contextlib import ExitStack

        with ExitStack() as ctx:
            cpool = ctx.enter_context(tc.tile_pool(name="consts", bufs=1))
            gipool = ctx.enter_context(tc.tile_pool(name="gi", bufs=1))
            hpool = ctx.enter_context(tc.tile_pool(name="hist", bufs=1))

            idx_sb = cpool.tile([128, NTILE], I32)
            ident = cpool.tile([128, 128], BF)
            BIAS = cpool.tile([128, 8], F32)
            B2N = cpool.tile([64, 128], F32)
            ONES1 = cpool.tile([64, B], F32)
            W_sb = cpool.tile([128, 768], F32)
            IB = cpool.tile([128, B], BF)
            BCRI = cpool.tile([128, 4, 128], BF)
            UT_sb = cpool.tile([128, 4, 768], BF)
            VT_sb = cpool.tile([128, 2, VS], BF)

            nc.sync.dma_start(idx_sb[:], idx_p[:, :])
            nc.sync.dma_start(BIAS[:], bias_p[:, :])
            nc.sync.dma_start(B2N[:], b2n_p[:, :])
            nc.sync.dma_start(W_sb[:], wt_p[:, :])
            nc.sync.dma_start(IB[:], ib_p[:, :])
            nc.sync.dma_start(BCRI[:], bcri_p[:, :].rearrange("p (g r) -> p g r", r=128))
            ut_src = ut_p[:, :].rearrange("(c p) f -> p c f", p=128)
            nc.sync.dma_start(UT_sb[:], ut_src)
            vt_src = vt_p[:, :].rearrange("(c p) f -> p c f", p=128)
            nc.sync.dma_start(VT_sb[:], vt_src)
            make_identity(nc, ident[:])
            nc.vector.memset(ONES1[:], 1.0)

            GIT = gipool.tile([128, NTILE, 4, 128], BF)
            GIN2 = gipool.tile([128, L, 2, B], BF)
            SUMS = cpool.tile([128, NPB * 8], F32)
            nc.vector.memset(SUMS[:], 0.0)

            H32 = hpool.tile([128, L, 2, B], F32)
            H_bf = hpool.tile([128, 2, NPOS], BF)
            nc.vector.memset(H32[:, 0, :, :], 0.0)

            import os
            _nrec = int(os.environ.get("NREC", str(L - 1)))
            do_front = "front" in phases
            do_rec = "rec" in phases

            ready_map = {}
            if "pass1" in phases and "rec" in phases:
                for p in range(NPB):
                    rdy = max(4 * p + 2, 126 - 4 * p)
                    ready_map.setdefault(rdy if INTERLEAVE_P1 else 126, []).append(p)

            dpool = ctx.enter_context(tc.tile_pool(name="dsmall", bufs=3))
            psd = ctx.enter_context(tc.tile_pool(name="psd", bufs=1, space="PSUM"))

            def emit_step(s):
                hf = H32[:, s, 0, :]
                hb = H32[:, s, 1, :]
                ps = psd.tile([128, 128], F32, tag="psri")
                psn = psd.tile([128, 64], F32, tag="psn")
                tbt = L - 1 - s
                for gidx, (tok, w0) in enumerate(
                    [(s, 0), (s, 128), (tbt, 384), (tbt, 512)]
                ):
                    jt, base = tok // 4, (tok % 4) * B
                    nc.tensor.matmul(
                        ps[:, gidx * B:(gidx + 1) * B],
                        GIT[base:base + B, jt, gidx, :],
                        IB[base:base + B, :],
                        start=True, stop=False,
                        tile_position=(base, 0),
                    )
                    h = hf if gidx < 2 else hb
                    nc.tensor.matmul(
                        ps[:, gidx * B:(gidx + 1) * B],
                        W_sb[:, w0:w0 + 128], h, start=False, stop=True,
                    )
                nc.tensor.matmul(
                    psn[:, 0:32], W_sb[:, 256:384], hf, start=True, stop=False
                )
                nc.tensor.matmul(
                    psn[:, 0:32], B2N[0:1, :], ONES1[0:1, :], start=False, stop=True
                )
                nc.tensor.matmul(
                    psn[:, 32:64], W_sb[:, 640:768], hb, start=True, stop=False
                )
                nc.tensor.matmul(
                    psn[:, 32:64], B2N[32:33, :], ONES1[32:33, :],
                    start=False, stop=True,
                )
                rz = dpool.tile([128, 2, 2, B], F32, tag="rz")
                nc.scalar.activation(rz[:], ps[:], AF.Tanh, scale=0.5)
                rview = rz[:, :, 0, :]
                zview = rz[:, :, 1, :]
                t1 = dpool.tile([128, 64], F32, tag="t1")
                nc.vector.scalar_tensor_tensor(
                    t1[:], rview, 1.0, psn[:], op0=ALU.add, op1=ALU.mult
                )
                t2 = dpool.tile([128, 64], F32, tag="t2")
                nc.vector.tensor_add(t2[:], t1[:], GIN2[:, s, :, :])
                q = dpool.tile([128, 64], F32, tag="q")
                nc.vector.scalar_tensor_tensor(
                    q[:], zview, 1.0, H32[:, s, :, :], op0=ALU.add, op1=ALU.mult
                )
                n = dpool.tile([128, 64], F32, tag="n")
                nc.scalar.activation(n[:], t2[:], AF.Tanh)
                u = dpool.tile([128, 64], F32, tag="u")
                nc.vector.scalar_tensor_tensor(
                    u[:], zview, 1.0, n[:], op0=ALU.subtract, op1=ALU.mult
                )
                d = dpool.tile([128, 64], F32, tag="d")
                nc.vector.tensor_sub(d[:], q[:], u[:])
                nc.vector.tensor_scalar_mul(H32[:, s + 1, :, :], d[:], 0.5)

            gate_cols = [(0, B_NF, False, 2), (1, B_NB, True, 5)]
            with (
                tc.tile_pool(name="front", bufs=4) as fpool,
                tc.tile_pool(name="et", bufs=1) as etpool,
                tc.tile_pool(name="pst", bufs=2, space="PSUM") as pst,
                tc.tile_pool(name="psg", bufs=2, space="PSUM") as psg,
            ):
                ET = etpool.tile([128, 4, NPOS], BF)

                def emit_chunk(ch):
                    for jj in range(4):
                        jt = ch * 4 + jj
                        et = fpool.tile([128, EMB], BF, tag="embtile")
                        nc.gpsimd.indirect_dma_start(
                            out=et[:],
                            out_offset=None,
                            in_=emb_p[:, :],
                            in_offset=bass.IndirectOffsetOnAxis(
                                ap=idx_sb[:, jt:jt + 1], axis=0
                            ),
                        )
                        for kc in range(4):
                            pt = pst.tile([128, 128], BF)
                            nc.tensor.transpose(
                                pt[:], et[:, kc * 128:(kc + 1) * 128], ident[:]
                            )
                            nc.scalar.activation(
                                ET[:, kc, jt * 128:(jt + 1) * 128], pt[:],
                                AF.Identity,
                            )
                    t0 = ch * 16
                    for gi, bcol, is_bwd, gcol in gate_cols:
                        ps = psg.tile([128, 512], F32)
                        for kc in range(4):
                            nc.tensor.matmul(
                                ps[:],
                                UT_sb[:, kc, gcol * 128:(gcol + 1) * 128],
                                ET[:, kc, ch * 512:(ch + 1) * 512],
                                start=(kc == 0),
                                stop=(kc == 3),
                            )
                        if is_bwd:
                            dst = GIN2[:, 112 - t0:128 - t0, gi, :][:, ::-1, :]
                        else:
                            dst = GIN2[:, t0:t0 + 16, gi, :]
                        nc.scalar.activation(
                            dst, ps[:].rearrange("p (t b) -> p t b", b=B),
                            AF.Identity, bias=BIAS[:, bcol:bcol + 1],
                        )
                    for gidx, gcol in enumerate([0, 1, 3, 4]):
                        for jj in range(4):
                            jt = ch * 4 + jj
                            ps = psg.tile([128, 128], F32, tag="psgit")
                            for kc in range(4):
                                nc.tensor.matmul(
                                    ps[:],
                                    ET[:, kc, jt * 128:(jt + 1) * 128],
                                    UT_sb[:, kc, gcol * 128:(gcol + 1) * 128],
                                    start=(kc == 0),
                                    stop=(kc == 3),
                                )
                            nc.vector.tensor_add(
                                GIT[:, jt, gidx, :], ps[:], BCRI[:, gidx, :]
                            )

                for pi, (ca, cb) in enumerate([(0, 7), (1, 6), (2, 5), (3, 4)]):
                    if do_front:
                        emit_chunk(ca)
                        emit_chunk(cb)
                    if do_rec:
                        for s in range(16 * pi, min(16 * (pi + 1), _nrec)):
                            emit_step(s)

            with (
                tc.tile_pool(name="pse", bufs=3, space="PSUM") as pse,
                tc.tile_pool(name="scr", bufs=3) as scrpool,
            ):

                def emit_pass1_pb(pb):
                    nc.vector.tensor_copy(
                        H_bf[:, 0, pb * 128:(pb + 1) * 128].rearrange(
                            "p (t b) -> p t b", b=B
                        ),
                        H32[:, 4 * pb:4 * pb + 4, 0, :],
                    )
                    nc.vector.tensor_copy(
                        H_bf[:, 1, pb * 128:(pb + 1) * 128].rearrange(
                            "p (t b) -> p t b", b=B
                        ),
                        H32[:, 124 - 4 * pb:128 - 4 * pb, 1, :][:, ::-1, :],
                    )
                    ready = max(4 * pb + 2, 126 - 4 * pb)
                    late = (not INTERLEAVE_P1) or ready >= 108
                    ndve = 3 if late else 0
                    for vt in range(NVT):
                        w = LAST_W if vt == NVT - 1 else EWIDTH
                        c0 = vt * EWIDTH
                        ps = pse.tile([128, EWIDTH], F32, tag="pse")
                        for half in range(0, w, 512):
                            hw = min(512, w - half)
                            for k in range(2):
                                nc.tensor.matmul(
                                    ps[:, half:half + hw],
                                    H_bf[:, k, pb * 128:(pb + 1) * 128],
                                    VT_sb[:, k, c0 + half:c0 + half + hw],
                                    start=(k == 0),
                                    stop=(k == 1),
                                )
                        slot = SUMS[:, pb * 8 + vt:pb * 8 + vt + 1]
                        if vt < NVT - ndve:
                            scr = scrpool.tile([128, EWIDTH], BF, tag="scr")
                            nc.scalar.activation(
                                scr[:, 0:w], ps[:, 0:w], AF.Exp, accum_out=slot
                            )
                        else:
                            it = scrpool.tile([128, EWIDTH], I32, tag="scri")
                            nc.vector.tensor_scalar(
                                it[:, 0:w], ps[:, 0:w], SCH_A, SCH_B,
                                op0=ALU.mult, op1=ALU.add,
                            )
                            nc.vector.tensor_reduce(
                                slot, it[:, 0:w].bitcast(F32),
                                axis=mybir.AxisListType.X, op=ALU.add,
                            )

                if do_rec:
                    for s in range(64, _nrec):
                        emit_step(s)
                        for p in ready_map.get(s, []):
                            emit_pass1_pb(p)
                if "pass1" in phases and not do_rec:
                    for pb in range(NPB):
                        emit_pass1_pb(pb)

                GA = list(range(5, 27))
                GB = list(range(0, 5)) + list(range(27, 32))
                negL = cpool.tile([128, NPB], F32)
                negpad = cpool.tile([128, 1], F32)
                nc.vector.memset(negpad[:], -float(PAD_COLS) * PADEXP)

                def emit_norm(group, cc_i, cc_o):
                    n = len(group)
                    S_g = cpool.tile([128, n], F32, name=f"S_{cc_i.name}", tag=f"sg{cc_i.name}")
                    if group == GA:
                        nc.vector.tensor_reduce(
                            S_g[:],
                            SUMS[:, 5 * 8:27 * 8].rearrange("p (a b) -> p a b", b=8),
                            axis=mybir.AxisListType.X, op=ALU.add,
                        )
                    else:
                        nc.vector.tensor_reduce(
                            S_g[:, 0:5],
                            SUMS[:, 0:5 * 8].rearrange("p (a b) -> p a b", b=8),
                            axis=mybir.AxisListType.X, op=ALU.add,
                        )
                        nc.vector.tensor_reduce(
                            S_g[:, 5:10],
                            SUMS[:, 27 * 8:32 * 8].rearrange("p (a b) -> p a b", b=8),
                            axis=mybir.AxisListType.X, op=ALU.add,
                        )
                    nc.sync.dma_start(cc_i[:, :], S_g[:])
                    nc.gpsimd.collective_compute(
                        "AllReduce", ALU.add,
                        replica_groups=[list(range(NCORES))],
                        ins=[cc_i[:, :].opt()], outs=[cc_o[:, :].opt()],
                    )
                    S_r = cpool.tile([128, n], F32, name=f"Sr_{cc_i.name}", tag=f"sr{cc_i.name}")
                    nc.sync.dma_start(S_r[:], cc_o[:, :])
                    lg = cpool.tile([128, n], F32, name=f"lg_{cc_i.name}", tag=f"lg{cc_i.name}")
                    nc.scalar.activation(lg[:], S_r[:], AF.Ln, bias=negpad[:])
                    for j, pb in enumerate(group):
                        pass
                    if group == GA:
                        nc.vector.tensor_scalar_mul(negL[:, 5:27], lg[:], -1.0)
                    else:
                        nc.vector.tensor_scalar_mul(negL[:, 0:5], lg[:, 0:5], -1.0)
                        nc.vector.tensor_scalar_mul(negL[:, 27:32], lg[:, 5:10], -1.0)

                def emit_pass2_pb(pb):
                    stg = stpool.tile([128, VS], out_dt, tag="stage")
                    for vt in range(NVT):
                        w = LAST_W if vt == NVT - 1 else EWIDTH
                        c0 = vt * EWIDTH
                        ps = pse.tile([128, EWIDTH], F32, tag="pse")
                        for half in range(0, w, 512):
                            hw = min(512, w - half)
                            for k in range(2):
                                nc.tensor.matmul(
                                    ps[:, half:half + hw],
                                    H_bf[:, k, pb * 128:(pb + 1) * 128],
                                    VT_sb[:, k, c0 + half:c0 + half + hw],
                                    start=(k == 0),
                                    stop=(k == 1),
                                )
                        if vt % 2 == 0:
                            nc.scalar.activation(
                                stg[:, c0:c0 + w], ps[:, 0:w], AF.Identity,
                                bias=negL[:, pb:pb + 1],
                            )
                        else:
                            nc.vector.tensor_scalar_add(
                                stg[:, c0:c0 + w], ps[:, 0:w], negL[:, pb:pb + 1],
                            )
                    nc.sync.dma_start(out_p[pb * 128:(pb + 1) * 128, :], stg[:])

                if "ar" in phases:
                    with tc.tile_pool(name="stage", bufs=2) as stpool:
                        emit_norm(GA, cc_inA, cc_outA)
                        if "pass2" in phases:
                            for pb in GA:
                                emit_pass2_pb(pb)
                        emit_norm(GB, cc_inB, cc_outB)
                        if "pass2" in phases:
                            for pb in GB:
                                emit_pass2_pb(pb)
                        nc.sync.dma_start(nls_p[:, :], negL[:])

    nc.finalize()
    return nc


_cache = {}


def _get_nc():
    if "nc" not in _cache:
        _cache["nc"] = build()
    return _cache["nc"]


def _host_prep(inputs):
    bf16 = ml_dtypes.bfloat16
    idx = np.ascontiguousarray(
        inputs["input_batch"].astype(np.int32).reshape(NPOS).reshape(NTILE, 128).T
    )
    emb_bf = inputs["embedding"].astype(bf16)
    ut = np.ascontiguousarray(
        np.concatenate([inputs["U"], inputs["U_b"]], axis=0).T
    ).astype(bf16)
    wt = np.ascontiguousarray(
        np.concatenate([inputs["W"], inputs["W_b"]], axis=0).T
    ).astype(np.float32)
    wt[:, 256:384] *= 0.5
    wt[:, 640:768] *= 0.5

    b1, b2 = inputs["bias_1"], inputs["bias_2"]
    b1b, b2b = inputs["bias_1_b"], inputs["bias_2_b"]
    bias = np.zeros((128, 8), np.float32)
    bias[:, B_RF] = b1[0:128] + b2[0:128]
    bias[:, B_IF] = b1[128:256] + b2[128:256]
    bias[:, B_RB] = b1b[0:128] + b2b[0:128]
    bias[:, B_IB] = b1b[128:256] + b2b[128:256]
    bias[:, B_NF] = b1[256:384]
    bias[:, B_NB] = b1b[256:384]
    bias[:, B2NF] = b2[256:384]
    bias[:, B2NB] = b2b[256:384]
    b2nrow = np.zeros((64, 128), np.float32)
    b2nrow[0] = 0.5 * b2[256:384]
    b2nrow[32] = 0.5 * b2b[256:384]

    ib = np.tile(np.eye(B, dtype=np.float32), (4, 1)).astype(bf16)
    bcri = np.zeros((128, 512), np.float32)
    bcri[:, 0:128] = bias[:, B_RF]
    bcri[:, 128:256] = bias[:, B_IF]
    bcri[:, 256:384] = bias[:, B_RB]
    bcri[:, 384:512] = bias[:, B_IB]
    bcri = bcri.astype(bf16)

    vt_full = np.zeros((2 * REC, VPAD), np.float32)
    vt_full[:, :VOCAB] = inputs["V"].T
    vt_bf = vt_full.astype(bf16)

    in_maps = []
    for c in range(NCORES):
        in_maps.append(
            {
                "idx": idx,
                "emb": emb_bf,
                "ut": ut,
                "wt": wt,
                "bias": bias,
                "b2nrow": b2nrow,
                "ib": ib,
                "bcri": bcri,
                "vt": np.ascontiguousarray(vt_bf[:, c * VS:(c + 1) * VS]),
            }
        )
    return in_maps


def kernel(**inputs):
    from concourse.bass_utils import run_bass_kernel_spmd

    nc = _get_nc()
    in_maps = _host_prep(inputs)
    res = run_bass_kernel_spmd(nc, in_maps, core_ids=list(range(NCORES)))
    out = np.empty((NPOS, VPAD), np.float32)
    for c in range(NCORES):
        out[:, c * VS:(c + 1) * VS] = res.results[c]["out"].astype(np.float32)
    return out[:, :VOCAB].reshape(L, B, VOCAB)



# revision 42
# speedup vs baseline: 1.2647x; 1.2647x over previous
import numpy as np
import ml_dtypes

import concourse.bass as bass
import concourse.tile as tile
from concourse import mybir, bacc
from concourse.masks import make_identity

L, B, EMB, REC = 128, 32, 512, 128
VOCAB = 50257
NCORES = 8
VS = 6283
VPAD = VS * NCORES
PAD_COLS = VPAD - VOCAB
NPOS = L * B
NTILE = NPOS // 128
NPB = 32
EW = 512
NVT = 13
LAST_W = VS - (NVT - 1) * EW

SCALE_V = 64.0
SCALE_H = 16.0
DESC = 1.0 / (SCALE_V * SCALE_H)

SCH_A = float(np.float32(2.0**23 / np.log(2.0)))
SCH_B = float(np.float32((127 << 23) - 482619))
SCH_A16 = float(np.float32(2.0**7 / np.log(2.0)))
SCH_B16 = float(np.float32((127 << 7) - 7 + 0.5))
PADEXP16 = float(
    np.array([int(SCH_B16)], np.int16).view(ml_dtypes.bfloat16)[0]
)
LN_A = float(np.float32(2.0**23 / np.log(2.0)))
LN_B = 1064866805.0

C_P1A = 0.56
C_P1D = 1.05
C_P1D_LAST = 0.40
C_P2A = 0.52
C_P2D = 0.58
BUD_ACT = 1.12
BUD_DVE = 1.00

BF = mybir.dt.bfloat16
F8 = mybir.dt.float8e4
F32 = mybir.dt.float32
I16 = mybir.dt.int16
I32 = mybir.dt.int32
AF = mybir.ActivationFunctionType
ALU = mybir.AluOpType
AXX = mybir.AxisListType.X
DR = mybir.MatmulPerfMode.DoubleRow

B_RF, B_IF, B_RB, B_IB, B_NF, B_NB, B2NF, B2NB = range(8)

GROUPS = [
    list(range(14, 18)),
    list(range(11, 14)) + list(range(18, 21)),
    list(range(8, 11)) + list(range(21, 24)),
    list(range(5, 8)) + list(range(24, 27)),
    list(range(2, 5)) + list(range(27, 30)),
    list(range(0, 2)) + list(range(30, 32)),
]


def _runs(blks):
    runs = []
    for b in blks:
        if runs and b == runs[-1][0] + runs[-1][1]:
            runs[-1][1] += 1
        else:
            runs.append([b, 1])
    return [tuple(r) for r in runs]


def build(phases=("front", "rec", "pass1", "ar", "pass2")):
    nc = bacc.Bacc(num_swdge_queues=4)

    idx_p = nc.declare_dram_parameter("idx", [128, NTILE], I32, isOutput=False)
    emb_p = nc.declare_dram_parameter("emb", [VOCAB, EMB], BF, isOutput=False)
    ut_p = nc.declare_dram_parameter("ut", [EMB, 768], BF, isOutput=False)
    wt_p = nc.declare_dram_parameter("wt", [REC, 768], F32, isOutput=False)
    bias_p = nc.declare_dram_parameter("bias", [128, 8], F32, isOutput=False)
    b2n_p = nc.declare_dram_parameter("b2nrow", [64, 128], F32, isOutput=False)
    vt_p = nc.declare_dram_parameter("vt", [2 * REC, VS], F8, isOutput=False)
    ib_p = nc.declare_dram_parameter("ib", [128, B], BF, isOutput=False)
    bcri_p = nc.declare_dram_parameter("bcri", [128, 512], BF, isOutput=False)
    out_p = nc.declare_dram_parameter("out", [NPOS, VS], BF, isOutput=True)

    cc_in = []
    cc_out = []
    for g, blks in enumerate(GROUPS):
        n = len(blks)
        cc_in.append(nc.dram_tensor(f"cc_in{g}", [128, n], F32))
        cc_out.append(nc.dram_tensor(f"cc_out{g}", [128 * NCORES, n], F32))

    with tile.TileContext(nc) as tc:
        from contextlib import ExitStack

        with ExitStack() as ctx:
            cpool = ctx.enter_context(tc.tile_pool(name="consts", bufs=1))
            gipool = ctx.enter_context(tc.tile_pool(name="gi", bufs=1))
            hpool = ctx.enter_context(tc.tile_pool(name="hist", bufs=1))

            idx_sb = cpool.tile([128, NTILE], I32)
            ident = cpool.tile([128, 128], BF)
            BIAS = cpool.tile([128, 8], F32)
            B2N = cpool.tile([64, 128], F32)
            ONES1 = cpool.tile([64, B], F32)
            W_sb = cpool.tile([128, 768], F32)
            IB = cpool.tile([128, B], BF)
            BCRI = cpool.tile([128, 4, 128], BF)
            UT_sb = cpool.tile([128, 4, 768], BF)
            VT_sb = cpool.tile([128, 2, VS + 1], F8)

            nc.sync.dma_start(idx_sb[:], idx_p[:, :])
            nc.sync.dma_start(BIAS[:], bias_p[:, :])
            nc.sync.dma_start(B2N[:], b2n_p[:, :])
            nc.sync.dma_start(W_sb[:], wt_p[:, :])
            nc.sync.dma_start(IB[:], ib_p[:, :])
            nc.sync.dma_start(BCRI[:], bcri_p[:, :].rearrange("p (g r) -> p g r", r=128))
            ut_src = ut_p[:, :].rearrange("(c p) f -> p c f", p=128)
            nc.sync.dma_start(UT_sb[:], ut_src)
            vt_src = vt_p[:, :].rearrange("(c p) f -> p c f", p=128)
            nc.sync.dma_start(VT_sb[:, :, 0:VS], vt_src)
            make_identity(nc, ident[:])
            nc.vector.memset(ONES1[:], 1.0)

            GIT = gipool.tile([128, NTILE, 4, 128], BF)
            GIN2 = gipool.tile([128, L, 2, B], BF)
            SUMS = cpool.tile([128, NPB * 16], F32)
            nc.vector.memset(SUMS[:], 0.0)
            negL = cpool.tile([128, NPB], F32)

            H32 = hpool.tile([128, L, 2, B], F32)
            H_f8 = hpool.tile([128, 2, NPOS], F8)
            nc.vector.memset(H32[:, 0, :, :], 0.0)

            import os
            _nrec = int(os.environ.get("NREC", str(L - 1)))
            do_front = "front" in phases
            do_rec = "rec" in phases

            dpool = ctx.enter_context(tc.tile_pool(name="dsmall", bufs=3))
            psd = ctx.enter_context(tc.tile_pool(name="psd", bufs=1, space="PSUM"))

            def emit_step(s):
                hf = H32[:, s, 0, :]
                hb = H32[:, s, 1, :]
                ps = psd.tile([128, 128], F32, tag="psri")
                psn = psd.tile([128, 64], F32, tag="psn")
                tbt = L - 1 - s
                for gidx, (tok, w0) in enumerate(
                    [(s, 0), (s, 128), (tbt, 384), (tbt, 512)]
                ):
                    jt, base = tok // 4, (tok % 4) * B
                    nc.tensor.matmul(
                        ps[:, gidx * B:(gidx + 1) * B],
                        GIT[base:base + B, jt, gidx, :],
                        IB[base:base + B, :],
                        start=True, stop=False,
                        tile_position=(base, 0),
                    )
                    h = hf if gidx < 2 else hb
                    nc.tensor.matmul(
                        ps[:, gidx * B:(gidx + 1) * B],
                        W_sb[:, w0:w0 + 128], h, start=False, stop=True,
                    )
                nc.tensor.matmul(
                    psn[:, 0:32], W_sb[:, 256:384], hf, start=True, stop=False
                )
                nc.tensor.matmul(
                    psn[:, 0:32], B2N[0:1, :], ONES1[0:1, :], start=False, stop=True
                )
                nc.tensor.matmul(
                    psn[:, 32:64], W_sb[:, 640:768], hb, start=True, stop=False
                )
                nc.tensor.matmul(
                    psn[:, 32:64], B2N[32:33, :], ONES1[32:33, :],
                    start=False, stop=True,
                )
                rz = dpool.tile([128, 2, 2, B], F32, tag="rz")
                nc.scalar.activation(rz[:], ps[:], AF.Tanh, scale=0.5)
                rview = rz[:, :, 0, :]
                zview = rz[:, :, 1, :]
                t1 = dpool.tile([128, 64], F32, tag="t1")
                nc.vector.scalar_tensor_tensor(
                    t1[:], rview, 1.0, psn[:], op0=ALU.add, op1=ALU.mult
                )
                t2 = dpool.tile([128, 64], F32, tag="t2")
                nc.gpsimd.tensor_add(t2[:], t1[:], GIN2[:, s, :, :])
                n = dpool.tile([128, 64], F32, tag="n")
                nc.scalar.activation(n[:], t2[:], AF.Tanh)
                zh = dpool.tile([128, 64], F32, tag="zh")
                nc.vector.tensor_scalar(
                    zh[:], zview, 0.5, -0.5, op0=ALU.mult, op1=ALU.add
                )
                q = dpool.tile([128, 64], F32, tag="q")
                nc.vector.scalar_tensor_tensor(
                    q[:], zview, 1.0, H32[:, s, :, :], op0=ALU.add, op1=ALU.mult
                )
                u = dpool.tile([128, 64], F32, tag="u")
                nc.vector.tensor_tensor(u[:], zh[:], n[:], op=ALU.mult)
                nc.vector.scalar_tensor_tensor(
                    H32[:, s + 1, :, :], q[:], 0.5, u[:],
                    op0=ALU.mult, op1=ALU.subtract,
                )

            gate_cols = [(0, B_NF, False, 2), (1, B_NB, True, 5)]
            with (
                tc.tile_pool(name="front", bufs=4) as fpool,
                tc.tile_pool(name="et", bufs=1) as etpool,
                tc.tile_pool(name="pst", bufs=2, space="PSUM") as pst,
                tc.tile_pool(name="psg", bufs=2, space="PSUM") as psg,
            ):
                ET = etpool.tile([128, 4, NPOS], BF)

                def emit_chunk(ch):
                    for jj in range(4):
                        jt = ch * 4 + jj
                        et = fpool.tile([128, EMB], BF, tag="embtile")
                        nc.gpsimd.indirect_dma_start(
                            out=et[:],
                            out_offset=None,
                            in_=emb_p[:, :],
                            in_offset=bass.IndirectOffsetOnAxis(
                                ap=idx_sb[:, jt:jt + 1], axis=0
                            ),
                        )
                        for kc in range(4):
                            pt = pst.tile([128, 128], BF)
                            nc.tensor.transpose(
                                pt[:], et[:, kc * 128:(kc + 1) * 128], ident[:]
                            )
                            if kc % 2 == 0:
                                nc.scalar.activation(
                                    ET[:, kc, jt * 128:(jt + 1) * 128], pt[:],
                                    AF.Identity,
                                )
                            else:
                                nc.vector.tensor_copy(
                                    ET[:, kc, jt * 128:(jt + 1) * 128], pt[:]
                                )
                    t0 = ch * 16
                    for gi, bcol, is_bwd, gcol in gate_cols:
                        ps = psg.tile([128, 512], F32)
                        for kc in range(4):
                            nc.tensor.matmul(
                                ps[:],
                                UT_sb[:, kc, gcol * 128:(gcol + 1) * 128],
                                ET[:, kc, ch * 512:(ch + 1) * 512],
                                start=(kc == 0),
                                stop=(kc == 3),
                            )
                        if is_bwd:
                            dst = GIN2[:, 112 - t0:128 - t0, gi, :][:, ::-1, :]
                        else:
                            dst = GIN2[:, t0:t0 + 16, gi, :]
                        nc.scalar.activation(
                            dst, ps[:].rearrange("p (t b) -> p t b", b=B),
                            AF.Identity, bias=BIAS[:, bcol:bcol + 1],
                        )
                    for gidx, gcol in enumerate([0, 1, 3, 4]):
                        for jj in range(4):
                            jt = ch * 4 + jj
                            ps = psg.tile([128, 128], F32, tag="psgit")
                            for kc in range(4):
                                nc.tensor.matmul(
                                    ps[:],
                                    ET[:, kc, jt * 128:(jt + 1) * 128],
                                    UT_sb[:, kc, gcol * 128:(gcol + 1) * 128],
                                    start=(kc == 0),
                                    stop=(kc == 3),
                                )
                            nc.vector.tensor_add(
                                GIT[:, jt, gidx, :], ps[:], BCRI[:, gidx, :]
                            )

                for pi, (ca, cb) in enumerate([(0, 7), (1, 6), (2, 5), (3, 4)]):
                    if do_front:
                        emit_chunk(ca)
                        emit_chunk(cb)
                    if do_rec:
                        for s in range(16 * pi, min(16 * (pi + 1), min(_nrec, 64))):
                            emit_step(s)

            with (
                tc.tile_pool(name="pse", bufs=6, space="PSUM") as pse,
                tc.tile_pool(name="scr", bufs=3) as scrpool,
                tc.tile_pool(name="stage", bufs=6) as stpool,
            ):
                from collections import deque

                do_p1 = "pass1" in phases
                do_ar = "ar" in phases and do_p1
                do_p2 = "pass2" in phases and do_ar

                grp_of = {}
                for g, blks in enumerate(GROUPS):
                    for pb in blks:
                        grp_of[pb] = g
                rem = [len(blks) * NVT for blks in GROUPS]
                ag_step = [None] * len(GROUPS)
                fin_q = deque()
                p1q = deque()
                p2q = deque()
                p2_stg = {}
                clock = [0.0]

                ready_steps = {}
                for pb in range(NPB):
                    ready_steps.setdefault(
                        max(4 * pb + 2, 126 - 4 * pb), []
                    ).append(pb)

                def emit_casts(pb):
                    nc.vector.tensor_scalar_mul(
                        H_f8[:, 0, pb * 128:(pb + 1) * 128].rearrange(
                            "p (t b) -> p t b", b=B
                        ),
                        H32[:, 4 * pb:4 * pb + 4, 0, :], SCALE_H,
                    )
                    nc.vector.tensor_scalar_mul(
                        H_f8[:, 1, pb * 128:(pb + 1) * 128].rearrange(
                            "p (t b) -> p t b", b=B
                        ),
                        H32[:, 124 - 4 * pb:128 - 4 * pb, 1, :][:, ::-1, :],
                        SCALE_H,
                    )

                def mm_tile(pb, vt):
                    w = LAST_W if vt == NVT - 1 else EW
                    c0 = vt * EW
                    lhs = H_f8[:, :, pb * 128:(pb + 1) * 128]
                    ps = pse.tile([128, EW], F32, tag="pse")
                    nc.tensor.matmul(
                        ps[:, 0:w], lhs, VT_sb[:, :, c0:c0 + w],
                        start=True, stop=True, perf_mode=DR,
                    )
                    return ps, w, c0

                def emit_p1_tile(pb, vt, eng):
                    ps, w, _ = mm_tile(pb, vt)
                    slot = SUMS[:, pb * 16 + vt:pb * 16 + vt + 1]
                    if eng == "A":
                        scr = scrpool.tile([128, EW], BF, tag="scrA")
                        nc.scalar.activation(
                            scr[:, 0:w], ps[:, 0:w], AF.Exp,
                            scale=DESC, accum_out=slot,
                        )
                    else:
                        it = scrpool.tile([128, EW], I16, tag="scrD")
                        nc.vector.tensor_scalar(
                            it[:, 0:w], ps[:, 0:w], SCH_A16 * DESC, SCH_B16,
                            op0=ALU.mult, op1=ALU.add,
                        )
                        nc.vector.tensor_reduce(
                            slot, it[:, 0:w].bitcast(BF), axis=AXX, op=ALU.add
                        )

                def emit_p2_tile(pb, vt, eng):
                    if pb not in p2_stg:
                        p2_stg[pb] = stpool.tile(
                            [128, VS], BF, name=f"stg{pb}", tag="stage"
                        )
                    stg = p2_stg[pb]
                    ps, w, c0 = mm_tile(pb, vt)
                    nl = negL[:, pb:pb + 1]
                    if eng == "A":
                        nc.scalar.activation(
                            stg[:, c0:c0 + w], ps[:, 0:w], AF.Identity,
                            scale=DESC, bias=nl,
                        )
                    else:
                        nc.vector.tensor_scalar(
                            stg[:, c0:c0 + w], ps[:, 0:w], DESC, nl,
                            op0=ALU.mult, op1=ALU.add,
                        )
                    if vt == NVT - 1:
                        nc.sync.dma_start(
                            out_p[pb * 128:(pb + 1) * 128, :], stg[:]
                        )
                        del p2_stg[pb]

                def emit_ag(g):
                    blks = GROUPS[g]
                    n = len(blks)
                    Sg = cpool.tile([128, n], F32, name=f"sg{g}", tag=f"sg{g}")
                    off = 0
                    for b0, cnt in _runs(blks):
                        nc.vector.tensor_reduce(
                            Sg[:, off:off + cnt],
                            SUMS[:, b0 * 16:(b0 + cnt) * 16].rearrange(
                                "p (a b) -> p a b", b=16
                            ),
                            axis=AXX, op=ALU.add,
                        )
                        off += cnt
                    nc.scalar.dma_start(cc_in[g][:, :], Sg[:])
                    nc.gpsimd.collective_compute(
                        "AllGather", ALU.bypass,
                        replica_groups=[list(range(NCORES))],
                        ins=[cc_in[g][:, :].opt()],
                        outs=[cc_out[g][:, :].opt()],
                    )
                    ag_step[g] = clock[0]
                    fin_q.append(g)

                def emit_fin(g):
                    blks = GROUPS[g]
                    n = len(blks)
                    t8 = cpool.tile([128, 8, n], F32, name=f"sa{g}", tag=f"sa{g}")
                    nc.scalar.dma_start(
                        t8[:],
                        cc_out[g][:, :].rearrange("(c p) n -> p c n", p=128),
                    )
                    wk = cpool.tile([128, 5, n], F32, name=f"nw{g}", tag=f"nw{g}")
                    Ssub, l0, ei, t, u = (wk[:, i, :] for i in range(5))
                    nc.vector.tensor_reduce(
                        Ssub, t8[:].rearrange("p c n -> p n c"),
                        axis=AXX, op=ALU.add,
                    )
                    nc.vector.tensor_scalar_add(
                        Ssub, Ssub, -float(PAD_COLS) * PADEXP16
                    )
                    nc.vector.tensor_scalar(
                        l0, Ssub.bitcast(I32), 1.0 / LN_A, -LN_B / LN_A,
                        op0=ALU.mult, op1=ALU.add,
                    )
                    nc.vector.tensor_scalar(
                        ei.bitcast(I32), l0, -SCH_A, SCH_B,
                        op0=ALU.mult, op1=ALU.add,
                    )
                    nc.vector.tensor_tensor(t, Ssub, ei, op=ALU.mult)
                    nc.vector.tensor_scalar(
                        u, t, -1.0, 1.0, op0=ALU.mult, op1=ALU.add
                    )
                    off = 0
                    for b0, cnt in _runs(blks):
                        nc.vector.tensor_sub(
                            negL[:, b0:b0 + cnt],
                            u[:, off:off + cnt], l0[:, off:off + cnt],
                        )
                        off += cnt
                    if do_p2:
                        for pb in blks:
                            for vt in range(NVT):
                                p2q.append((pb, vt))

                def after_p1_emit(pb):
                    g = grp_of[pb]
                    rem[g] -= 1
                    if rem[g] == 0 and do_ar:
                        emit_ag(g)

                def check_fins(force=False):
                    while fin_q and (force or clock[0] - ag_step[fin_q[0]] > 15.0):
                        emit_fin(fin_q.popleft())

                def step_interleave():
                    act_b, dve_b = BUD_ACT, BUD_DVE
                    while p1q:
                        pb, vt = p1q[0]
                        if vt == NVT - 1:
                            if dve_b < C_P1D_LAST:
                                break
                            p1q.popleft()
                            emit_p1_tile(pb, vt, "D")
                            dve_b -= C_P1D_LAST
                            clock[0] += C_P1D_LAST / 2
                        elif act_b >= C_P1A:
                            p1q.popleft()
                            emit_p1_tile(pb, vt, "A")
                            act_b -= C_P1A
                            clock[0] += C_P1A / 2
                        elif dve_b >= C_P1D:
                            p1q.popleft()
                            emit_p1_tile(pb, vt, "D")
                            dve_b -= C_P1D
                            clock[0] += C_P1D / 2
                        else:
                            break
                        after_p1_emit(pb)
                    while p2q:
                        if act_b >= C_P2A:
                            pb, vt = p2q.popleft()
                            emit_p2_tile(pb, vt, "A")
                            act_b -= C_P2A
                            clock[0] += C_P2A / 2
                        elif dve_b >= C_P2D:
                            pb, vt = p2q.popleft()
                            emit_p2_tile(pb, vt, "D")
                            dve_b -= C_P2D
                            clock[0] += C_P2D / 2
                        else:
                            break
                    check_fins()

                def tail_drain():
                    busy = {"A": 0.0, "D": 0.0}
                    flip = [True]
                    while p1q or p2q or fin_q:
                        check_fins()
                        flip[0] = not flip[0]
                        if p2q and (flip[0] or not p1q):
                            pb, vt = p2q.popleft()
                            if busy["A"] + C_P2A <= busy["D"] + C_P2D:
                                emit_p2_tile(pb, vt, "A")
                                busy["A"] += C_P2A
                                clock[0] += C_P2A / 2
                            else:
                                emit_p2_tile(pb, vt, "D")
                                busy["D"] += C_P2D
                                clock[0] += C_P2D / 2
                        elif p1q:
                            pb, vt = p1q.popleft()
                            if vt == NVT - 1:
                                emit_p1_tile(pb, vt, "D")
                                busy["D"] += C_P1D_LAST
                                clock[0] += C_P1D_LAST / 2
                            elif busy["A"] + C_P1A <= busy["D"] + C_P1D:
                                emit_p1_tile(pb, vt, "A")
                                busy["A"] += C_P1A
                                clock[0] += C_P1A / 2
                            else:
                                emit_p1_tile(pb, vt, "D")
                                busy["D"] += C_P1D
                                clock[0] += C_P1D / 2
                            after_p1_emit(pb)
                        elif fin_q:
                            check_fins(force=True)

                if do_rec:
                    for s in range(64, _nrec):
                        emit_step(s)
                        for pb in ready_steps.get(s - 1, []):
                            if do_p1:
                                for vt in range(NVT):
                                    p1q.append((pb, vt))
                        for pb in ready_steps.get(s, []):
                            if do_p1:
                                emit_casts(pb)
                        clock[0] += 1.7
                        step_interleave()
                    for pb in ready_steps.get(_nrec - 1, []):
                        if do_p1:
                            for vt in range(NVT):
                                p1q.append((pb, vt))
                else:
                    for s in sorted(ready_steps):
                        for pb in ready_steps[s]:
                            if do_p1:
                                emit_casts(pb)
                                for vt in range(NVT):
                                    p1q.append((pb, vt))
                if _nrec == L - 1:
                    tail_drain()

    nc.finalize()
    return nc


_cache = {}


def _get_nc():
    if "nc" not in _cache:
        _cache["nc"] = build()
    return _cache["nc"]


def _host_prep(inputs):
    bf16 = ml_dtypes.bfloat16
    fp8 = ml_dtypes.float8_e4m3
    idx = np.ascontiguousarray(
        inputs["input_batch"].astype(np.int32).reshape(NPOS).reshape(NTILE, 128).T
    )
    emb_bf = inputs["embedding"].astype(bf16)
    ut = np.ascontiguousarray(
        np.concatenate([inputs["U"], inputs["U_b"]], axis=0).T
    ).astype(bf16)
    wt = np.ascontiguousarray(
        np.concatenate([inputs["W"], inputs["W_b"]], axis=0).T
    ).astype(np.float32)
    wt[:, 256:384] *= 0.5
    wt[:, 640:768] *= 0.5

    b1, b2 = inputs["bias_1"], inputs["bias_2"]
    b1b, b2b = inputs["bias_1_b"], inputs["bias_2_b"]
    bias = np.zeros((128, 8), np.float32)
    bias[:, B_RF] = b1[0:128] + b2[0:128]
    bias[:, B_IF] = b1[128:256] + b2[128:256]
    bias[:, B_RB] = b1b[0:128] + b2b[0:128]
    bias[:, B_IB] = b1b[128:256] + b2b[128:256]
    bias[:, B_NF] = b1[256:384]
    bias[:, B_NB] = b1b[256:384]
    bias[:, B2NF] = b2[256:384]
    bias[:, B2NB] = b2b[256:384]
    b2nrow = np.zeros((64, 128), np.float32)
    b2nrow[0] = 0.5 * b2[256:384]
    b2nrow[32] = 0.5 * b2b[256:384]

    ib = np.tile(np.eye(B, dtype=np.float32), (4, 1)).astype(bf16)
    bcri = np.zeros((128, 512), np.float32)
    bcri[:, 0:128] = bias[:, B_RF]
    bcri[:, 128:256] = bias[:, B_IF]
    bcri[:, 256:384] = bias[:, B_RB]
    bcri[:, 384:512] = bias[:, B_IB]
    bcri = bcri.astype(bf16)

    vt_full = np.zeros((2 * REC, VPAD), np.float32)
    vt_full[:, :VOCAB] = inputs["V"].T * SCALE_V
    vt_f8 = np.clip(vt_full, -240.0, 240.0).astype(fp8)

    in_maps = []
    for c in range(NCORES):
        in_maps.append(
            {
                "idx": idx,
                "emb": emb_bf,
                "ut": ut,
                "wt": wt,
                "bias": bias,
                "b2nrow": b2nrow,
                "ib": ib,
                "bcri": bcri,
                "vt": np.ascontiguousarray(vt_f8[:, c * VS:(c + 1) * VS]),
            }
        )
    return in_maps


def kernel(**inputs):
    from concourse.bass_utils import run_bass_kernel_spmd

    nc = _get_nc()
    in_maps = _host_prep(inputs)
    res = run_bass_kernel_spmd(nc, in_maps, core_ids=list(range(NCORES)))
    out = np.empty((NPOS, VPAD), np.float32)
    for c in range(NCORES):
        out[:, c * VS:(c + 1) * VS] = res.results[c]["out"].astype(np.float32)
    return out[:, :VOCAB].reshape(L, B, VOCAB)


# revision 44
# speedup vs baseline: 1.2954x; 1.0242x over previous
import numpy as np
import ml_dtypes

import concourse.bass as bass
import concourse.tile as tile
from concourse import mybir, bacc
from concourse.masks import make_identity

L, B, EMB, REC = 128, 32, 512, 128
VOCAB = 50257
NCORES = 8
VS = 6283
VPAD = VS * NCORES
PAD_COLS = VPAD - VOCAB
NPOS = L * B
NTILE = NPOS // 128
NPB = 32
EW = 512
NVT = 13
LAST_W = VS - (NVT - 1) * EW

SCALE_V = 64.0
SCALE_H = 16.0
DESC = 1.0 / (SCALE_V * SCALE_H)

SCH_A = float(np.float32(2.0**23 / np.log(2.0)))
SCH_B = float(np.float32((127 << 23) - 482619))
SCH_A16 = float(np.float32(2.0**7 / np.log(2.0)))
SCH_B16 = float(np.float32((127 << 7) - 7 + 0.5))
PADEXP16 = float(
    np.array([int(SCH_B16)], np.int16).view(ml_dtypes.bfloat16)[0]
)
LN_A = float(np.float32(2.0**23 / np.log(2.0)))
LN_B = 1064866805.0

C_P1A = 0.56
C_P1D = 1.05
C_P1D_LAST = 0.40
C_P2A = 0.52
C_P2D = 0.58
BUD_ACT = 1.12
BUD_DVE = 1.80
FIN_HOLD = 15.0

BF = mybir.dt.bfloat16
F8 = mybir.dt.float8e4
F32 = mybir.dt.float32
I16 = mybir.dt.int16
I32 = mybir.dt.int32
AF = mybir.ActivationFunctionType
ALU = mybir.AluOpType
AXX = mybir.AxisListType.X
DR = mybir.MatmulPerfMode.DoubleRow

B_RF, B_IF, B_RB, B_IB, B_NF, B_NB, B2NF, B2NB = range(8)

GROUPS = [
    list(range(13, 19)),
    list(range(9, 13)) + list(range(19, 23)),
    list(range(4, 9)) + list(range(23, 28)),
    list(range(0, 4)) + list(range(28, 32)),
]


def _runs(blks):
    runs = []
    for b in blks:
        if runs and b == runs[-1][0] + runs[-1][1]:
            runs[-1][1] += 1
        else:
            runs.append([b, 1])
    return [tuple(r) for r in runs]


def build(phases=("front", "rec", "pass1", "ar", "pass2")):
    nc = bacc.Bacc(num_swdge_queues=4)

    idx_p = nc.declare_dram_parameter("idx", [128, NTILE], I32, isOutput=False)
    emb_p = nc.declare_dram_parameter("emb", [VOCAB, EMB], BF, isOutput=False)
    ut_p = nc.declare_dram_parameter("ut", [EMB, 768], BF, isOutput=False)
    wt_p = nc.declare_dram_parameter("wt", [REC, 768], F32, isOutput=False)
    bias_p = nc.declare_dram_parameter("bias", [128, 8], F32, isOutput=False)
    b2n_p = nc.declare_dram_parameter("b2nrow", [64, 128], F32, isOutput=False)
    vt_p = nc.declare_dram_parameter("vt", [2 * REC, VS], F8, isOutput=False)
    ib_p = nc.declare_dram_parameter("ib", [128, B], BF, isOutput=False)
    bcri_p = nc.declare_dram_parameter("bcri", [128, 512], BF, isOutput=False)
    out_p = nc.declare_dram_parameter("out", [NPOS, VS], BF, isOutput=True)

    cc_in = []
    cc_out = []
    for g, blks in enumerate(GROUPS):
        n = len(blks)
        cc_in.append(nc.dram_tensor(f"cc_in{g}", [128, n], F32))
        cc_out.append(nc.dram_tensor(f"cc_out{g}", [128 * NCORES, n], F32))

    with tile.TileContext(nc) as tc:
        from contextlib import ExitStack

        with ExitStack() as ctx:
            cpool = ctx.enter_context(tc.tile_pool(name="consts", bufs=1))
            gipool = ctx.enter_context(tc.tile_pool(name="gi", bufs=1))
            hpool = ctx.enter_context(tc.tile_pool(name="hist", bufs=1))

            idx_sb = cpool.tile([128, NTILE], I32)
            ident = cpool.tile([128, 128], BF)
            BIAS = cpool.tile([128, 8], F32)
            B2N = cpool.tile([64, 128], F32)
            ONES1 = cpool.tile([64, B], F32)
            W_sb = cpool.tile([128, 768], F32)
            IB = cpool.tile([128, B], BF)
            BCRI = cpool.tile([128, 4, 128], BF)
            UT_sb = cpool.tile([128, 4, 768], BF)
            VT_sb = cpool.tile([128, 2, VS + 1], F8)

            nc.sync.dma_start(idx_sb[:], idx_p[:, :])
            nc.sync.dma_start(BIAS[:], bias_p[:, :])
            nc.sync.dma_start(B2N[:], b2n_p[:, :])
            nc.sync.dma_start(W_sb[:], wt_p[:, :])
            nc.sync.dma_start(IB[:], ib_p[:, :])
            nc.sync.dma_start(BCRI[:], bcri_p[:, :].rearrange("p (g r) -> p g r", r=128))
            ut_src = ut_p[:, :].rearrange("(c p) f -> p c f", p=128)
            nc.sync.dma_start(UT_sb[:], ut_src)
            vt_src = vt_p[:, :].rearrange("(c p) f -> p c f", p=128)
            nc.sync.dma_start(VT_sb[:, :, 0:VS], vt_src)
            make_identity(nc, ident[:])
            nc.vector.memset(ONES1[:], 1.0)

            GIT = gipool.tile([128, NTILE, 4, 128], BF)
            GIN2 = gipool.tile([128, L, 2, B], BF)
            SUMS = cpool.tile([128, NPB * 16], F32)
            nc.vector.memset(SUMS[:], 0.0)
            negL = cpool.tile([128, NPB], F32)

            H32 = hpool.tile([128, L, 2, B], F32)
            H_f8 = hpool.tile([128, 2, NPOS], F8)
            nc.vector.memset(H32[:, 0, :, :], 0.0)

            import os
            _nrec = int(os.environ.get("NREC", str(L - 1)))
            do_front = "front" in phases
            do_rec = "rec" in phases

            dpool = ctx.enter_context(tc.tile_pool(name="dsmall", bufs=3))
            psd = ctx.enter_context(tc.tile_pool(name="psd", bufs=1, space="PSUM"))

            def emit_step(s):
                hf = H32[:, s, 0, :]
                hb = H32[:, s, 1, :]
                ps = psd.tile([128, 128], F32, tag="psri")
                psn = psd.tile([128, 64], F32, tag="psn")
                tbt = L - 1 - s
                for gidx, (tok, w0) in enumerate(
                    [(s, 0), (s, 128), (tbt, 384), (tbt, 512)]
                ):
                    jt, base = tok // 4, (tok % 4) * B
                    nc.tensor.matmul(
                        ps[:, gidx * B:(gidx + 1) * B],
                        GIT[base:base + B, jt, gidx, :],
                        IB[base:base + B, :],
                        start=True, stop=False,
                        tile_position=(base, 0),
                    )
                    h = hf if gidx < 2 else hb
                    nc.tensor.matmul(
                        ps[:, gidx * B:(gidx + 1) * B],
                        W_sb[:, w0:w0 + 128], h, start=False, stop=True,
                    )
                nc.tensor.matmul(
                    psn[:, 0:32], W_sb[:, 256:384], hf, start=True, stop=False
                )
                nc.tensor.matmul(
                    psn[:, 0:32], B2N[0:1, :], ONES1[0:1, :], start=False, stop=True
                )
                nc.tensor.matmul(
                    psn[:, 32:64], W_sb[:, 640:768], hb, start=True, stop=False
                )
                nc.tensor.matmul(
                    psn[:, 32:64], B2N[32:33, :], ONES1[32:33, :],
                    start=False, stop=True,
                )
                rz = dpool.tile([128, 2, 2, B], F32, tag="rz")
                nc.scalar.activation(rz[:], ps[:], AF.Tanh, scale=0.5)
                rview = rz[:, :, 0, :]
                zview = rz[:, :, 1, :]
                t1 = dpool.tile([128, 64], F32, tag="t1")
                nc.vector.scalar_tensor_tensor(
                    t1[:], rview, 1.0, psn[:], op0=ALU.add, op1=ALU.mult
                )
                t2 = dpool.tile([128, 64], F32, tag="t2")
                nc.gpsimd.tensor_add(t2[:], t1[:], GIN2[:, s, :, :])
                n = dpool.tile([128, 64], F32, tag="n")
                nc.scalar.activation(n[:], t2[:], AF.Tanh)
                zh = dpool.tile([128, 64], F32, tag="zh")
                nc.vector.tensor_scalar(
                    zh[:], zview, 0.5, -0.5, op0=ALU.mult, op1=ALU.add
                )
                q = dpool.tile([128, 64], F32, tag="q")
                nc.vector.scalar_tensor_tensor(
                    q[:], zview, 1.0, H32[:, s, :, :], op0=ALU.add, op1=ALU.mult
                )
                u = dpool.tile([128, 64], F32, tag="u")
                nc.vector.tensor_tensor(u[:], zh[:], n[:], op=ALU.mult)
                nc.vector.scalar_tensor_tensor(
                    H32[:, s + 1, :, :], q[:], 0.5, u[:],
                    op0=ALU.mult, op1=ALU.subtract,
                )

            gate_cols = [(0, B_NF, False, 2), (1, B_NB, True, 5)]
            with (
                tc.tile_pool(name="front", bufs=4) as fpool,
                tc.tile_pool(name="et", bufs=1) as etpool,
                tc.tile_pool(name="pst", bufs=2, space="PSUM") as pst,
                tc.tile_pool(name="psg", bufs=2, space="PSUM") as psg,
            ):
                ET = etpool.tile([128, 4, NPOS], BF)

                def emit_chunk(ch):
                    for jj in range(4):
                        jt = ch * 4 + jj
                        et = fpool.tile([128, EMB], BF, tag="embtile")
                        nc.gpsimd.indirect_dma_start(
                            out=et[:],
                            out_offset=None,
                            in_=emb_p[:, :],
                            in_offset=bass.IndirectOffsetOnAxis(
                                ap=idx_sb[:, jt:jt + 1], axis=0
                            ),
                        )
                        for kc in range(4):
                            pt = pst.tile([128, 128], BF)
                            nc.tensor.transpose(
                                pt[:], et[:, kc * 128:(kc + 1) * 128], ident[:]
                            )
                            if kc % 2 == 0:
                                nc.scalar.activation(
                                    ET[:, kc, jt * 128:(jt + 1) * 128], pt[:],
                                    AF.Identity,
                                )
                            else:
                                nc.vector.tensor_copy(
                                    ET[:, kc, jt * 128:(jt + 1) * 128], pt[:]
                                )
                    t0 = ch * 16
                    for gi, bcol, is_bwd, gcol in gate_cols:
                        ps = psg.tile([128, 512], F32)
                        for kc in range(4):
                            nc.tensor.matmul(
                                ps[:],
                                UT_sb[:, kc, gcol * 128:(gcol + 1) * 128],
                                ET[:, kc, ch * 512:(ch + 1) * 512],
                                start=(kc == 0),
                                stop=(kc == 3),
                            )
                        if is_bwd:
                            dst = GIN2[:, 112 - t0:128 - t0, gi, :][:, ::-1, :]
                        else:
                            dst = GIN2[:, t0:t0 + 16, gi, :]
                        nc.scalar.activation(
                            dst, ps[:].rearrange("p (t b) -> p t b", b=B),
                            AF.Identity, bias=BIAS[:, bcol:bcol + 1],
                        )
                    for gidx, gcol in enumerate([0, 1, 3, 4]):
                        for jj in range(4):
                            jt = ch * 4 + jj
                            ps = psg.tile([128, 128], F32, tag="psgit")
                            for kc in range(4):
                                nc.tensor.matmul(
                                    ps[:],
                                    ET[:, kc, jt * 128:(jt + 1) * 128],
                                    UT_sb[:, kc, gcol * 128:(gcol + 1) * 128],
                                    start=(kc == 0),
                                    stop=(kc == 3),
                                )
                            nc.vector.tensor_add(
                                GIT[:, jt, gidx, :], ps[:], BCRI[:, gidx, :]
                            )

                for pi, (ca, cb) in enumerate([(0, 7), (1, 6), (2, 5), (3, 4)]):
                    if do_front:
                        emit_chunk(ca)
                        emit_chunk(cb)
                    if do_rec:
                        for s in range(16 * pi, min(16 * (pi + 1), min(_nrec, 64))):
                            emit_step(s)

            with (
                tc.tile_pool(name="pse", bufs=6, space="PSUM") as pse,
                tc.tile_pool(name="scr", bufs=3) as scrpool,
                tc.tile_pool(name="stage", bufs=6) as stpool,
            ):
                from collections import deque

                do_p1 = "pass1" in phases
                do_ar = "ar" in phases and do_p1
                do_p2 = "pass2" in phases and do_ar

                grp_of = {}
                for g, blks in enumerate(GROUPS):
                    for pb in blks:
                        grp_of[pb] = g
                rem = [len(blks) * NVT for blks in GROUPS]
                ag_step = [None] * len(GROUPS)
                fin_q = deque()
                p1q = deque()
                p2q = deque()
                p2_stg = {}
                clock = [0.0]

                ready_steps = {}
                for pb in range(NPB):
                    ready_steps.setdefault(
                        max(4 * pb + 2, 126 - 4 * pb), []
                    ).append(pb)

                def emit_casts(pb):
                    nc.vector.tensor_scalar_mul(
                        H_f8[:, 0, pb * 128:(pb + 1) * 128].rearrange(
                            "p (t b) -> p t b", b=B
                        ),
                        H32[:, 4 * pb:4 * pb + 4, 0, :], SCALE_H,
                    )
                    nc.vector.tensor_scalar_mul(
                        H_f8[:, 1, pb * 128:(pb + 1) * 128].rearrange(
                            "p (t b) -> p t b", b=B
                        ),
                        H32[:, 124 - 4 * pb:128 - 4 * pb, 1, :][:, ::-1, :],
                        SCALE_H,
                    )

                def mm_tile(pb, vt):
                    w = LAST_W if vt == NVT - 1 else EW
                    c0 = vt * EW
                    lhs = H_f8[:, :, pb * 128:(pb + 1) * 128]
                    ps = pse.tile([128, EW], F32, tag="pse")
                    nc.tensor.matmul(
                        ps[:, 0:w], lhs, VT_sb[:, :, c0:c0 + w],
                        start=True, stop=True, perf_mode=DR,
                    )
                    return ps, w, c0

                def emit_p1_tile(pb, vt, eng):
                    ps, w, _ = mm_tile(pb, vt)
                    slot = SUMS[:, pb * 16 + vt:pb * 16 + vt + 1]
                    if eng == "A":
                        scr = scrpool.tile([128, EW], BF, tag="scrA")
                        nc.scalar.activation(
                            scr[:, 0:w], ps[:, 0:w], AF.Exp,
                            scale=DESC, accum_out=slot,
                        )
                    else:
                        it = scrpool.tile([128, EW], I16, tag="scrD")
                        nc.vector.tensor_scalar(
                            it[:, 0:w], ps[:, 0:w], SCH_A16 * DESC, SCH_B16,
                            op0=ALU.mult, op1=ALU.add,
                        )
                        nc.vector.tensor_reduce(
                            slot, it[:, 0:w].bitcast(BF), axis=AXX, op=ALU.add
                        )

                def emit_p2_tile(pb, vt, eng):
                    if pb not in p2_stg:
                        p2_stg[pb] = stpool.tile(
                            [128, VS], BF, name=f"stg{pb}", tag="stage"
                        )
                    stg = p2_stg[pb]
                    ps, w, c0 = mm_tile(pb, vt)
                    nl = negL[:, pb:pb + 1]
                    if eng == "A":
                        nc.scalar.activation(
                            stg[:, c0:c0 + w], ps[:, 0:w], AF.Identity,
                            scale=DESC, bias=nl,
                        )
                    else:
                        nc.vector.tensor_scalar(
                            stg[:, c0:c0 + w], ps[:, 0:w], DESC, nl,
                            op0=ALU.mult, op1=ALU.add,
                        )
                    if vt == NVT - 1:
                        nc.sync.dma_start(
                            out_p[pb * 128:(pb + 1) * 128, :], stg[:]
                        )
                        del p2_stg[pb]

                def emit_ag(g):
                    blks = GROUPS[g]
                    n = len(blks)
                    Sg = cpool.tile([128, n], F32, name=f"sg{g}", tag=f"sg{g}")
                    off = 0
                    for b0, cnt in _runs(blks):
                        nc.vector.tensor_reduce(
                            Sg[:, off:off + cnt],
                            SUMS[:, b0 * 16:(b0 + cnt) * 16].rearrange(
                                "p (a b) -> p a b", b=16
                            ),
                            axis=AXX, op=ALU.add,
                        )
                        off += cnt
                    nc.scalar.dma_start(cc_in[g][:, :], Sg[:])
                    nc.gpsimd.collective_compute(
                        "AllGather", ALU.bypass,
                        replica_groups=[list(range(NCORES))],
                        ins=[cc_in[g][:, :].opt()],
                        outs=[cc_out[g][:, :].opt()],
                    )
                    ag_step[g] = clock[0]
                    fin_q.append(g)

                def emit_fin(g):
                    blks = GROUPS[g]
                    n = len(blks)
                    t8 = cpool.tile([128, 8, n], F32, name=f"sa{g}", tag=f"sa{g}")
                    nc.scalar.dma_start(
                        t8[:],
                        cc_out[g][:, :].rearrange("(c p) n -> p c n", p=128),
                    )
                    wk = cpool.tile([128, 5, n], F32, name=f"nw{g}", tag=f"nw{g}")
                    Ssub, l0, ei, t, u = (wk[:, i, :] for i in range(5))
                    nc.vector.tensor_reduce(
                        Ssub, t8[:].rearrange("p c n -> p n c"),
                        axis=AXX, op=ALU.add,
                    )
                    nc.vector.tensor_scalar_add(
                        Ssub, Ssub, -float(PAD_COLS) * PADEXP16
                    )
                    nc.vector.tensor_scalar(
                        l0, Ssub.bitcast(I32), 1.0 / LN_A, -LN_B / LN_A,
                        op0=ALU.mult, op1=ALU.add,
                    )
                    nc.vector.tensor_scalar(
                        ei.bitcast(I32), l0, -SCH_A, SCH_B,
                        op0=ALU.mult, op1=ALU.add,
                    )
                    nc.vector.tensor_tensor(t, Ssub, ei, op=ALU.mult)
                    nc.vector.tensor_scalar(
                        u, t, -1.0, 1.0, op0=ALU.mult, op1=ALU.add
                    )
                    off = 0
                    for b0, cnt in _runs(blks):
                        nc.vector.tensor_sub(
                            negL[:, b0:b0 + cnt],
                            u[:, off:off + cnt], l0[:, off:off + cnt],
                        )
                        off += cnt
                    if do_p2:
                        for pb in blks:
                            for vt in range(NVT):
                                p2q.append((pb, vt))

                def after_p1_emit(pb):
                    g = grp_of[pb]
                    rem[g] -= 1
                    if rem[g] == 0 and do_ar:
                        emit_ag(g)

                def check_fins(force=False):
                    while fin_q and (force or clock[0] - ag_step[fin_q[0]] > FIN_HOLD):
                        emit_fin(fin_q.popleft())

                def step_interleave():
                    act_b, dve_b = BUD_ACT, BUD_DVE
                    while p1q:
                        pb, vt = p1q[0]
                        if vt == NVT - 1:
                            if dve_b < C_P1D_LAST:
                                break
                            p1q.popleft()
                            emit_p1_tile(pb, vt, "D")
                            dve_b -= C_P1D_LAST
                            clock[0] += C_P1D_LAST / 2
                        elif act_b >= C_P1A:
                            p1q.popleft()
                            emit_p1_tile(pb, vt, "A")
                            act_b -= C_P1A
                            clock[0] += C_P1A / 2
                        elif dve_b >= C_P1D:
                            p1q.popleft()
                            emit_p1_tile(pb, vt, "D")
                            dve_b -= C_P1D
                            clock[0] += C_P1D / 2
                        else:
                            break
                        after_p1_emit(pb)
                    while p2q:
                        if act_b >= C_P2A:
                            pb, vt = p2q.popleft()
                            emit_p2_tile(pb, vt, "A")
                            act_b -= C_P2A
                            clock[0] += C_P2A / 2
                        elif dve_b >= C_P2D:
                            pb, vt = p2q.popleft()
                            emit_p2_tile(pb, vt, "D")
                            dve_b -= C_P2D
                            clock[0] += C_P2D / 2
                        else:
                            break
                    check_fins()

                def tail_drain():
                    busy = {"A": 0.0, "D": 0.0}
                    flip = [True]
                    while p1q or p2q or fin_q:
                        check_fins()
                        flip[0] = not flip[0]
                        if p2q and (flip[0] or not p1q):
                            pb, vt = p2q.popleft()
                            if busy["A"] + C_P2A <= busy["D"] + C_P2D:
                                emit_p2_tile(pb, vt, "A")
                                busy["A"] += C_P2A
                                clock[0] += C_P2A / 2
                            else:
                                emit_p2_tile(pb, vt, "D")
                                busy["D"] += C_P2D
                                clock[0] += C_P2D / 2
                        elif p1q:
                            pb, vt = p1q.popleft()
                            if vt == NVT - 1:
                                emit_p1_tile(pb, vt, "D")
                                busy["D"] += C_P1D_LAST
                                clock[0] += C_P1D_LAST / 2
                            elif busy["A"] + C_P1A <= busy["D"] + C_P1D:
                                emit_p1_tile(pb, vt, "A")
                                busy["A"] += C_P1A
                                clock[0] += C_P1A / 2
                            else:
                                emit_p1_tile(pb, vt, "D")
                                busy["D"] += C_P1D
                                clock[0] += C_P1D / 2
                            after_p1_emit(pb)
                        elif fin_q:
                            check_fins(force=True)

                if do_rec:
                    for s in range(64, _nrec):
                        emit_step(s)
                        for pb in ready_steps.get(s - 1, []):
                            if do_p1:
                                for vt in range(NVT):
                                    p1q.append((pb, vt))
                        for pb in ready_steps.get(s, []):
                            if do_p1:
                                emit_casts(pb)
                        clock[0] += 1.7
                        step_interleave()
                    for pb in ready_steps.get(_nrec - 1, []):
                        if do_p1:
                            for vt in range(NVT):
                                p1q.append((pb, vt))
                else:
                    for s in sorted(ready_steps):
                        for pb in ready_steps[s]:
                            if do_p1:
                                emit_casts(pb)
                                for vt in range(NVT):
                                    p1q.append((pb, vt))
                if _nrec == L - 1:
                    tail_drain()

    nc.finalize()
    return nc


_cache = {}


def _get_nc():
    if "nc" not in _cache:
        _cache["nc"] = build()
    return _cache["nc"]


def _host_prep(inputs):
    bf16 = ml_dtypes.bfloat16
    fp8 = ml_dtypes.float8_e4m3
    idx = np.ascontiguousarray(
        inputs["input_batch"].astype(np.int32).reshape(NPOS).reshape(NTILE, 128).T
    )
    emb_bf = inputs["embedding"].astype(bf16)
    ut = np.ascontiguousarray(
        np.concatenate([inputs["U"], inputs["U_b"]], axis=0).T
    ).astype(bf16)
    wt = np.ascontiguousarray(
        np.concatenate([inputs["W"], inputs["W_b"]], axis=0).T
    ).astype(np.float32)
    wt[:, 256:384] *= 0.5
    wt[:, 640:768] *= 0.5

    b1, b2 = inputs["bias_1"], inputs["bias_2"]
    b1b, b2b = inputs["bias_1_b"], inputs["bias_2_b"]
    bias = np.zeros((128, 8), np.float32)
    bias[:, B_RF] = b1[0:128] + b2[0:128]
    bias[:, B_IF] = b1[128:256] + b2[128:256]
    bias[:, B_RB] = b1b[0:128] + b2b[0:128]
    bias[:, B_IB] = b1b[128:256] + b2b[128:256]
    bias[:, B_NF] = b1[256:384]
    bias[:, B_NB] = b1b[256:384]
    bias[:, B2NF] = b2[256:384]
    bias[:, B2NB] = b2b[256:384]
    b2nrow = np.zeros((64, 128), np.float32)
    b2nrow[0] = 0.5 * b2[256:384]
    b2nrow[32] = 0.5 * b2b[256:384]

    ib = np.tile(np.eye(B, dtype=np.float32), (4, 1)).astype(bf16)
    bcri = np.zeros((128, 512), np.float32)
    bcri[:, 0:128] = bias[:, B_RF]
    bcri[:, 128:256] = bias[:, B_IF]
    bcri[:, 256:384] = bias[:, B_RB]
    bcri[:, 384:512] = bias[:, B_IB]
    bcri = bcri.astype(bf16)

    vt_full = np.zeros((2 * REC, VPAD), np.float32)
    vt_full[:, :VOCAB] = inputs["V"].T * SCALE_V
    vt_f8 = np.clip(vt_full, -240.0, 240.0).astype(fp8)

    in_maps = []
    for c in range(NCORES):
        in_maps.append(
            {
                "idx": idx,
                "emb": emb_bf,
                "ut": ut,
                "wt": wt,
                "bias": bias,
                "b2nrow": b2nrow,
                "ib": ib,
                "bcri": bcri,
                "vt": np.ascontiguousarray(vt_f8[:, c * VS:(c + 1) * VS]),
            }
        )
    return in_maps


def kernel(**inputs):
    from concourse.bass_utils import run_bass_kernel_spmd

    nc = _get_nc()
    in_maps = _host_prep(inputs)
    res = run_bass_kernel_spmd(nc, in_maps, core_ids=list(range(NCORES)))
    out = np.empty((NPOS, VPAD), np.float32)
    for c in range(NCORES):
        out[:, c * VS:(c + 1) * VS] = res.results[c]["out"].astype(np.float32)
    return out[:, :VOCAB].reshape(L, B, VOCAB)
